# revision 5
# baseline (speedup 1.0000x reference)
"""Bass/Trainium2 kernel for BertSelfAttention (B=4, S=2048, D=1024, H=16).

Sharding: 8 cores = 4 batches x 2 head-groups (8 heads / 512 channels each).
Each core computes q/k/v projections for its head-group, attention scores,
softmax (no max-subtraction needed: |scores| <~ 6), attention probs (fp16),
context, and a partial output projection. Host sums the two partial outputs
per batch and upcasts attn to fp32.

Device dataflow per core (all matmul inputs fp16, fp32 PSUM accumulate):
  qT[c,s] = (Wq/8 @ hs.T), kT[c,s] = Wk @ hs.T, v[s,c] = hs @ Wv.T
  qT/kT are stored per-head as [66, 2048] tiles: rows 0-63 the head's
  channels, row 64 carries the mask trick (q side: ones; k side:
  -60*(1-mask), so masked scores get -60 inside the matmul and exp
  underflows to exact fp16 zero), row 65 carries the bq-bias correction
  (q side: ones; k side: (bq.k + bq.bk)/8 computed via a matvec).  The
  q.bk term is constant per query row and softmax-invariant, so dropped.
  scores psum[q,k] -> exp on ACT (fused row-sum accum) -> fp16
  -> normalize with per-partition reciprocal (DVE) -> DMA to attn output
  -> DMA transpose read-back [k,q] -> ctx^T psum = v.T @ attn^T
  -> out partial = ctx @ Wo_cols.T  (bias bo + Wo@bv added on host).
"""

import numpy as np

B, S, D, H = 4, 2048, 1024, 16
DK = D // H          # 64
NCORES = 8
HPG = 8              # heads per head-group (per core)
C = HPG * DK         # 512 local channels per core
KAUG = DK + 2        # 66: head channels + mask row + bias row

FP16 = np.float16

_compiled = {}


def _build_program():
    import concourse.tile as tile
    from concourse import bacc, mybir

    f16 = mybir.dt.float16
    f32 = mybir.dt.float32
    EXP = mybir.ActivationFunctionType.Exp
    IDENT = mybir.ActivationFunctionType.Identity
    ADD = mybir.AluOpType.add
    AXX = mybir.AxisListType.X

    nc = bacc.Bacc("TRN2", target_bir_lowering=False, debug=False,
                   num_devices=NCORES)

    # ---- DRAM I/O (per-core shards; same program on all 8 cores) ----
    hsT_d = nc.dram_tensor("hsT", [D, S], f16, kind="ExternalInput").ap()
    wq_d = nc.dram_tensor("wqT", [D, C], f16, kind="ExternalInput").ap()
    wk_d = nc.dram_tensor("wkT", [D, C], f16, kind="ExternalInput").ap()
    wv_d = nc.dram_tensor("wvT", [D, C], f16, kind="ExternalInput").ap()
    wo_d = nc.dram_tensor("woT", [C, D], f16, kind="ExternalInput").ap()
    mneg_d = nc.dram_tensor("maskneg", [1, S], f16, kind="ExternalInput").ap()
    bqs_d = nc.dram_tensor("bqs", [DK, HPG], f16, kind="ExternalInput").ap()
    bqbk_d = nc.dram_tensor("bqbk", [1, HPG], f32, kind="ExternalInput").ap()

    attn_d = nc.dram_tensor("attn", [HPG, S, S], f16, kind="ExternalOutput").ap()
    outp_d = nc.dram_tensor("outp", [S, D], f32, kind="ExternalOutput").ap()

    NQT = S // 128       # 16 query tiles of 128 rows
    NKC = S // 512       # 4 key chunks of 512
    NDC = D // 128       # 8 contraction chunks for projections
    NSC = S // 512       # 4 s-chunks for qT/kT N dim
    NVT = S // 128       # 16 s-tiles for v
    NCC = C // 128       # 4 c-chunks (2 heads each)

    with tile.TileContext(nc) as tc:
        # ---------- persistent SBUF ----------
        with tc.tile_pool(name="persist", bufs=1) as persist, \
             tc.tile_pool(name="psA", bufs=2, space="PSUM") as psA, \
             tc.tile_pool(name="psCtx", bufs=2, space="PSUM") as psCtx, \
             tc.tile_pool(name="psOut", bufs=2, space="PSUM") as psOut:

            qT = persist.tile([KAUG, HPG * S], f16, tag="qT")
            kT = persist.tile([KAUG, HPG * S], f16, tag="kT")
            v_sb = persist.tile([128, NVT * C], f16, tag="v")
            ctxT = persist.tile([128, NCC * S], f16, tag="ctxT")
            wo_sb = persist.tile([128, NCC * D], f16, tag="wo")
            bqs_sb = persist.tile([DK, HPG], f16, tag="bqs")
            bqbk_sb = persist.tile([1, HPG], f32, tag="bqbk")

            nc.sync.dma_start(bqs_sb[:], bqs_d[:])
            nc.sync.dma_start(bqbk_sb[:], bqbk_d[:])
            for ci in range(NCC):
                nc.sync.dma_start(wo_sb[:, ci * D:(ci + 1) * D],
                                  wo_d[ci * 128:(ci + 1) * 128, :])

            # ones rows for the mask / bias augmentation (q side)
            nc.gpsimd.memset(qT[DK:KAUG, :], 1.0)
            # mask row on k side: same host-prepared row for every head
            for h in range(HPG):
                nc.sync.dma_start(kT[DK:DK + 1, h * S:(h + 1) * S], mneg_d[:])

            # ---------- phase 1: projections ----------
            with tc.tile_pool(name="proj", bufs=1) as proj:
                hsT = proj.tile([128, NDC * S], f16, tag="hsT")
                wq_sb = proj.tile([128, NDC * C], f16, tag="wq")
                wk_sb = proj.tile([128, NDC * C], f16, tag="wk")
                wv_sb = proj.tile([128, NDC * C], f16, tag="wv")
                for di in range(NDC):
                    nc.sync.dma_start(hsT[:, di * S:(di + 1) * S],
                                      hsT_d[di * 128:(di + 1) * 128, :])
                    nc.sync.dma_start(wq_sb[:, di * C:(di + 1) * C],
                                      wq_d[di * 128:(di + 1) * 128, :])
                    nc.sync.dma_start(wk_sb[:, di * C:(di + 1) * C],
                                      wk_d[di * 128:(di + 1) * 128, :])
                    nc.sync.dma_start(wv_sb[:, di * C:(di + 1) * C],
                                      wv_d[di * 128:(di + 1) * 128, :])

                # qT / kT: out [c-chunk 128 (2 heads), s-chunk 512].
                # Engines cannot move data across partitions, so the odd
                # head's rows (psum partitions 64-127) bounce via an fp16
                # tile and an SBUF->SBUF DMA down to partitions 0-63.
                for w_sb, dst in ((wq_sb, qT), (wk_sb, kT)):
                    for mi in range(NCC):
                        for ni in range(NSC):
                            ps = psA.tile([128, 1024], f32, tag="mm")
                            pss = ps[:, 0:512]
                            for di in range(NDC):
                                nc.tensor.matmul(
                                    pss,
                                    lhsT=w_sb[:, di * C + mi * 128:
                                              di * C + (mi + 1) * 128],
                                    rhs=hsT[:, di * S + ni * 512:
                                            di * S + (ni + 1) * 512],
                                    start=(di == 0), stop=(di == NDC - 1))
                            he, ho = 2 * mi, 2 * mi + 1
                            nc.vector.tensor_copy(
                                dst[0:DK, he * S + ni * 512:
                                    he * S + (ni + 1) * 512],
                                pss[0:DK, :])
                            bt = proj.tile([128, 512], f16, tag="bounce",
                                           bufs=3)
                            nc.vector.tensor_copy(bt[DK:128, :],
                                                  pss[DK:128, :])
                            nc.sync.dma_start(
                                dst[0:DK, ho * S + ni * 512:
                                    ho * S + (ni + 1) * 512],
                                bt[DK:128, :])

                # v natural: out [s-tile 128, C]
                for mi in range(NVT):
                    ps = psA.tile([128, 1024], f32, tag="mm")
                    pss = ps[:, 0:512]
                    for di in range(NDC):
                        nc.tensor.matmul(
                            pss,
                            lhsT=hsT[:, di * S + mi * 128:
                                     di * S + (mi + 1) * 128],
                            rhs=wv_sb[:, di * C:(di + 1) * C],
                            start=(di == 0), stop=(di == NDC - 1))
                    nc.vector.tensor_copy(v_sb[:, mi * C:(mi + 1) * C], pss)

            # ---------- phase 1b: bias-correction row on k side ----------
            with tc.tile_pool(name="brow", bufs=2) as brow_pool:
                for h in range(HPG):
                    btile = brow_pool.tile([1, S], f16, tag="brow")
                    for ni in range(NSC):
                        ps = psA.tile([128, 1024], f32, tag="mm")
                        psr = ps[0:1, 0:512]
                        nc.tensor.matmul(
                            psr,
                            lhsT=bqs_sb[:, h:h + 1],
                            rhs=kT[0:DK, h * S + ni * 512:h * S + (ni + 1) * 512],
                            start=True, stop=True)
                        nc.scalar.activation(
                            btile[:, ni * 512:(ni + 1) * 512], psr,
                            IDENT, bias=bqbk_sb[:, h:h + 1])
                    nc.sync.dma_start(kT[DK + 1:KAUG, h * S:(h + 1) * S],
                                      btile[:])

            # ---------- phase 2: attention, per head-pair ----------
            with tc.tile_pool(name="soft", bufs=3) as soft, \
                 tc.tile_pool(name="sums", bufs=8) as sums_pool, \
                 tc.tile_pool(name="attnT", bufs=4) as attnT_pool:
                for hp in range(HPG // 2):
                    for h in (2 * hp, 2 * hp + 1):
                        for qt in range(NQT):
                            exp_t = soft.tile([128, S], f16, tag="exp")
                            sums = sums_pool.tile([128, 2], f32, tag="s4")
                            for kc in range(2):
                                ps = psA.tile([128, 1024], f32, tag="mm")
                                for j in range(2):
                                    nc.tensor.matmul(
                                        ps[:, j * 512:(j + 1) * 512],
                                        lhsT=qT[:, h * S + qt * 128:
                                                h * S + (qt + 1) * 128],
                                        rhs=kT[:, h * S + (2 * kc + j) * 512:
                                               h * S + (2 * kc + j + 1) * 512],
                                        start=True, stop=True)
                                nc.scalar.activation(
                                    exp_t[:, kc * 1024:(kc + 1) * 1024], ps[:],
                                    EXP, accum_out=sums[:, kc:kc + 1])
                            tot = sums_pool.tile([128, 1], f32, tag="tot")
                            nc.vector.tensor_reduce(tot[:], sums[:],
                                                    axis=AXX, op=ADD)
                            rec = sums_pool.tile([128, 1], f32, tag="rec")
                            nc.vector.reciprocal(rec[:], tot[:])
                            nc.vector.tensor_scalar_mul(exp_t[:], exp_t[:],
                                                        rec[:])
                            nc.sync.dma_start(
                                attn_d[h, qt * 128:(qt + 1) * 128, :],
                                exp_t[:])
                    # context for the pair: ctxT[c, q] += v.T @ attnT
                    for qg in range(4):
                        cps = psCtx.tile([128, 512], f32, tag="ctx")
                        for h in (2 * hp, 2 * hp + 1):
                            hh = h % 2
                            for kc in range(16):
                                att = attnT_pool.tile([128, 512], f16,
                                                      tag="attnT")
                                nc.sync.dma_start(
                                    att[:],
                                    attn_d[h, qg * 512:(qg + 1) * 512,
                                           kc * 128:(kc + 1) * 128],
                                    transpose=True)
                                nc.tensor.matmul(
                                    cps[hh * DK:(hh + 1) * DK, :],
                                    lhsT=v_sb[:, kc * C + h * DK:
                                              kc * C + (h + 1) * DK],
                                    rhs=att[:],
                                    start=(kc == 0), stop=(kc == 15))
                        nc.vector.tensor_copy(
                            ctxT[:, hp * S + qg * 512:hp * S + (qg + 1) * 512],
                            cps[:])

            # ---------- phase 3: output projection ----------
            with tc.tile_pool(name="outc", bufs=3) as outc:
                for sc in range(NQT):
                    for ec in range(2):
                        ops = psOut.tile([128, 512], f32, tag="out")
                        for ci in range(NCC):
                            nc.tensor.matmul(
                                ops[:],
                                lhsT=ctxT[:, ci * S + sc * 128:
                                          ci * S + (sc + 1) * 128],
                                rhs=wo_sb[:, ci * D + ec * 512:
                                          ci * D + (ec + 1) * 512],
                                start=(ci == 0), stop=(ci == NCC - 1))
                        ot = outc.tile([128, 512], f32, tag="ot")
                        nc.vector.tensor_copy(ot[:], ops[:])
                        nc.sync.dma_start(
                            outp_d[sc * 128:(sc + 1) * 128,
                                   ec * 512:(ec + 1) * 512],
                            ot[:])

    nc.compile()
    return nc


def _get_program():
    if "nc" not in _compiled:
        _compiled["nc"] = _build_program()
    return _compiled["nc"]


def kernel(hidden_states, attention_mask, Wq, bq, Wk, bk, Wv, bv, Wo, bo):
    out, attn, _ = _run(hidden_states, attention_mask, Wq, bq, Wk, bk,
                        Wv, bv, Wo, bo)
    return out, attn


def _run(hidden_states, attention_mask, Wq, bq, Wk, bk, Wv, bv, Wo, bo,
         trace=False):
    import sys
    if "/opt/trn_rl_repo" not in sys.path:
        sys.path.insert(0, "/opt/trn_rl_repo")
    from concourse.bass_utils import run_bass_kernel_spmd

    hidden_states = np.asarray(hidden_states, dtype=np.float32)
    attention_mask = np.asarray(attention_mask)
    Wq = np.asarray(Wq, dtype=np.float32)
    Wk = np.asarray(Wk, dtype=np.float32)
    Wv = np.asarray(Wv, dtype=np.float32)
    Wo = np.asarray(Wo, dtype=np.float32)
    bq = np.asarray(bq, dtype=np.float32)
    bk = np.asarray(bk, dtype=np.float32)
    bv = np.asarray(bv, dtype=np.float32)
    bo = np.asarray(bo, dtype=np.float32)

    nc = _get_program()

    scale = 1.0 / np.sqrt(DK)
    mask = attention_mask.reshape(B, S).astype(np.float32)
    in_maps = []
    for core in range(NCORES):
        b, hg = divmod(core, 2)
        sl = slice(hg * C, (hg + 1) * C)
        bq_s = (bq[sl] * scale).astype(FP16)
        bk_s = bk[sl].astype(np.float32)
        bqbk = np.zeros((1, HPG), np.float32)
        for h in range(HPG):
            bqbk[0, h] = float(
                np.dot(bq[sl][h * DK:(h + 1) * DK],
                       bk[sl][h * DK:(h + 1) * DK]) * scale)
        del bk_s
        in_maps.append({
            "hsT": np.ascontiguousarray(hidden_states[b].T).astype(FP16),
            "wqT": np.ascontiguousarray((Wq[sl, :] * scale).T).astype(FP16),
            "wkT": np.ascontiguousarray(Wk[sl, :].T).astype(FP16),
            "wvT": np.ascontiguousarray(Wv[sl, :].T).astype(FP16),
            "woT": np.ascontiguousarray(Wo[:, sl].T).astype(FP16),
            "maskneg": ((mask[b] - 1.0) * 60.0).astype(FP16).reshape(1, S),
            "bqs": np.ascontiguousarray(bq_s.reshape(HPG, DK).T),
            "bqbk": bqbk,
        })

    res = run_bass_kernel_spmd(nc, in_maps, list(range(NCORES)), trace=trace)

    attn = np.empty((B, H, S, S), dtype=np.float32)
    out = np.zeros((B, S, D), dtype=np.float32)
    for core in range(NCORES):
        b, hg = divmod(core, 2)
        attn[b, hg * HPG:(hg + 1) * HPG] = res.results[core]["attn"]
        out[b] += res.results[core]["outp"]
    out += (Wo @ bv + bo)
    return out, attn, res
